# revision 23
# baseline (speedup 1.0000x reference)
"""Segment-softmax additive-attention pooling on 8 TRN2 NeuronCores.

Math (per node n with segment b = batch_index[n]):
    beta[n]  = v . tanh(Q@W + K@U)[n]
    alpha[n] = exp(beta[n]) / sum_{m in b} exp(beta[m])
    out[b]   = sum_{n in b} alpha[n] * V[n]

Strategy:
  - batch_index is sorted -> shard rows across 8 cores at segment
    boundaries (no cross-core reduction, <=79 segments per core).
  - Host pre-transposes Q,K to [D, rows], block-transposes V to a
    [128, tiles*129] layout with a column of ones baked in after each
    128-wide V block (gives the softmax denominator for free), and
    reformats batch_index to per-tile f32 columns. Every DMA is then
    fully contiguous per partition and batched to ~2MB transfers.
  - Compute dtype fp16 (same bytes/rate as bf16, 8x the mantissa).
  - On device, per 1024-row slab (software-pipelined across slabs):
      S^T = W^T Q^T + U^T K^T        (PE, 4 matmuls, rhs free=512)
      T^T = tanh(S^T)                (ACT, one op per slab)
      beta = T @ v                   (PE, 8 matmuls of rhs free=1)
      e = exp(beta)                  (ACT)
      A[n,j] = (bic[n]==iota[j]) * e[n]   (DVE, j over NB_LOC=4 local
        slots only: a 1024-row slab of sorted batch_index spans <= 3
        segments + padding, so the one-hot can be 4 wide, not 80)
      stage[j,:] += A^T @ [V | 1]    (PE, per-slab [4,129] PSUM stage)
      stage -> stage_sb[4s:4s+4]     (DVE copy to an SBUF stage array)
  - Final: numg = R^T @ stage_sb  (2 fp32 matmuls with a host-built
    one-hot routing matrix R mapping (slab, local slot) -> segment),
    then out[j,:] = numg[j,0:128] / max(numg[j,128],1e-30), DMA out.
  - Scatter-add is expressed as matmul with a one-hot-weighted A, so no
    indirect addressing at all; softmax normalization is folded into a
    single division at the end.
"""

import numpy as np

N_CORES = 8
D = 128
NUM_SEGMENTS = 512
SLABW = 1024
NSLAB = 63
R_PAD = NSLAB * SLABW        # 64512 padded rows per core
T_TILES = R_PAD // 128       # 504 tiles of 128 rows
G_TILES = SLABW // 128       # 8 row-tiles per slab
NB = 80                      # local segment slots per core (partition dim)
PAD_SLOT = NB - 1            # numg row that padding rows are routed to

DT_NAME = "float16"          # compute dtype for Q/K/V/W/U/v ("float32"|"bfloat16")

_compiled = {}
LAST_RESULT = None


def _build_nc(dt_name, nslab=NSLAB, nb_loc=4):
    import concourse.bass as bass
    import concourse.bacc as bacc
    import concourse.tile as tile
    from concourse import mybir

    NSLAB_ = nslab
    NB_LOC = nb_loc
    R_PAD_ = NSLAB_ * SLABW
    T_TILES_ = R_PAD_ // 128
    # DMA load groups (in slabs): small first groups so compute starts
    # early, small last groups so the compute tail overlaps the final
    # transfers instead of draining serially after the last byte.
    import os as _os
    _lg = int(_os.environ.get("K_LOADG", "6"))
    front = [1, 1, 2]
    back = [2, 1, 1]
    group_sizes = []
    rem = NSLAB_
    for want in front:
        if rem > sum(back):
            g = min(want, rem - sum(back))
            group_sizes.append(g)
            rem -= g
    while rem > sum(back):
        g = min(_lg, rem - sum(back))
        group_sizes.append(g)
        rem -= g
    for want in back:
        if rem > 0:
            g = min(want, rem)
            group_sizes.append(g)
            rem -= g
    assert sum(group_sizes) == NSLAB_, (group_sizes, NSLAB_)
    LOADG_MAX = max(group_sizes)
    group_start = [0]
    for g in group_sizes:
        group_start.append(group_start[-1] + g)

    dt = getattr(mybir.dt, dt_name)
    f32 = mybir.dt.float32
    bf16 = mybir.dt.bfloat16
    nc = bacc.Bacc("TRN2", target_bir_lowering=False, debug=False,
                   num_devices=N_CORES)

    qt_d = nc.dram_tensor("qt", [128, R_PAD_], dt, kind="ExternalInput").ap()
    kt_d = nc.dram_tensor("kt", [128, R_PAD_], dt, kind="ExternalInput").ap()
    vr_d = nc.dram_tensor("vr", [128, T_TILES_ * 129], dt, kind="ExternalInput").ap()
    bic_d = nc.dram_tensor("bic", [128, T_TILES_], dt, kind="ExternalInput").ap()
    NQUAD = -(-NSLAB_ // 3)      # routing granularity: 3 slabs per group
    iota_d = nc.dram_tensor("iota", [128, NB_LOC], dt, kind="ExternalInput").ap()
    rmat_d = nc.dram_tensor("rmat", [128, NQUAD * NB], bf16,
                            kind="ExternalInput").ap()
    w_d = nc.dram_tensor("w", [128, 128], dt, kind="ExternalInput").ap()
    u_d = nc.dram_tensor("u", [128, 128], dt, kind="ExternalInput").ap()
    vv_d = nc.dram_tensor("vv", [128, 1], dt, kind="ExternalInput").ap()
    out_d = nc.dram_tensor("out", [NB, 128], f32, kind="ExternalOutput").ap()

    Tanh = mybir.ActivationFunctionType.Tanh
    Exp = mybir.ActivationFunctionType.Exp
    is_equal = mybir.AluOpType.is_equal
    mult = mybir.AluOpType.mult

    _bufs = int(_os.environ.get("K_BUFS", "4"))
    with tile.TileContext(nc) as tc, \
         tc.tile_pool(name="const", bufs=1) as constp, \
         tc.tile_pool(name="qk", bufs=_bufs) as qkp, \
         tc.tile_pool(name="vsl", bufs=_bufs) as vslp, \
         tc.tile_pool(name="tt", bufs=3) as ttp, \
         tc.tile_pool(name="sm", bufs=4) as smp, \
         tc.tile_pool(name="at", bufs=3) as atp, \
         tc.tile_pool(name="fin", bufs=1) as finp, \
         tc.tile_pool(name="ps_s", bufs=2, space="PSUM") as pss, \
         tc.tile_pool(name="ps_c", bufs=2, space="PSUM") as psc, \
         tc.tile_pool(name="ps_o", bufs=1, space="PSUM") as pso:

        wt = constp.tile([128, 128], dt)
        nc.sync.dma_start(out=wt, in_=w_d)
        ut = constp.tile([128, 128], dt)
        nc.sync.dma_start(out=ut, in_=u_d)
        vv = constp.tile([128, 1], dt)
        nc.scalar.dma_start(out=vv, in_=vv_d)
        iota = constp.tile([128, NB_LOC], dt)
        nc.scalar.dma_start(out=iota, in_=iota_d)
        bic = constp.tile([128, T_TILES_], dt)
        nc.scalar.dma_start(out=bic, in_=bic_d)
        rmat = constp.tile([128, NQUAD, NB], bf16)
        nc.scalar.dma_start(
            out=rmat,
            in_=rmat_d.rearrange("p (t g) -> p t g", g=NB))

        numg = pso.tile([NB, 129], f32)
        # two group-stage SBUF tiles: 3 slabs' [NB_LOC,129] stage blocks
        # at partition offsets 0/32/64; rows NB_LOC..31 of each block
        # stay zero (memset once) so the routing matmul sees clean zeros
        stg_a = finp.tile([128, 129], bf16, tag="sg0")
        stg_b = finp.tile([128, 129], bf16, tag="sg1")
        stg_grp = [stg_a, stg_b]
        nc.vector.memset(stg_grp[0], 0.0)
        nc.vector.memset(stg_grp[1], 0.0)

        # pipeline state per slab
        vr_s = [None] * NSLAB_
        st_s = [None] * NSLAB_
        tt_s = [None] * NSLAB_
        cb_s = [None] * NSLAB_   # combo PSUM: [:,0:8]=beta, [0:4,8:137]=stage
        eb_s = [None] * NSLAB_
        at_s = [None] * NSLAB_

        qt_g = [None]
        kt_g = [None]
        vr_g = [None]
        g_base = [0]

        def stage_load_group(g):
            s0 = group_start[g]
            ns = group_sizes[g]
            w0 = s0 * SLABW
            w1 = w0 + ns * SLABW
            qt_t = qkp.tile([128, LOADG_MAX * SLABW], dt, tag="qt")
            nc.sync.dma_start(out=qt_t[:, :w1 - w0], in_=qt_d[:, w0:w1])
            kt_t = qkp.tile([128, LOADG_MAX * SLABW], dt, tag="kt")
            nc.sync.dma_start(out=kt_t[:, :w1 - w0], in_=kt_d[:, w0:w1])
            t0 = s0 * G_TILES
            t1 = t0 + ns * G_TILES
            vr_t = vslp.tile([128, LOADG_MAX * G_TILES, 129], dt, tag="vr")
            nc.sync.dma_start(
                out=vr_t[:, :t1 - t0, :],
                in_=vr_d[:, t0 * 129:t1 * 129].rearrange(
                    "p (t d) -> p t d", d=129))
            qt_g[0], kt_g[0], vr_g[0] = qt_t, kt_t, vr_t
            g_base[0] = s0

        next_group = [0]

        def stage_load(s):
            if next_group[0] < len(group_start) - 1 and s == group_start[next_group[0]]:
                stage_load_group(next_group[0])
                next_group[0] += 1
            o = (s - g_base[0]) * SLABW
            vr_s[s] = vr_g[0][:, (s - g_base[0]) * G_TILES:
                              (s - g_base[0] + 1) * G_TILES, :]
            return (qt_g[0][:, o:o + SLABW], kt_g[0][:, o:o + SLABW])

        def stage_s(s, qt_t, kt_t):
            st = pss.tile([128, SLABW], f32, tag="st")
            for h in range(SLABW // 512):
                sl = slice(h * 512, (h + 1) * 512)
                nc.tensor.matmul(st[:, sl], lhsT=wt, rhs=qt_t[:, sl],
                                 start=True, stop=False)
            for h in range(SLABW // 512):
                sl = slice(h * 512, (h + 1) * 512)
                nc.tensor.matmul(st[:, sl], lhsT=ut, rhs=kt_t[:, sl],
                                 start=False, stop=True)
            st_s[s] = st

        def stage_tanh(s):
            tt = ttp.tile([128, SLABW], dt, tag="tt")
            nc.scalar.activation(out=tt, in_=st_s[s], func=Tanh)
            tt_s[s] = tt
            st_s[s] = None

        def stage_beta(s):
            cb = psc.tile([128, 137], f32, tag="cb")
            tt = tt_s[s]
            for t in range(G_TILES):
                nc.tensor.matmul(cb[:, t:t + 1],
                                 lhsT=tt[:, t * 128:(t + 1) * 128],
                                 rhs=vv, start=True, stop=True,
                                 skip_group_check=True)
            cb_s[s] = cb

        def stage_exp(s):
            eb = smp.tile([128, G_TILES], dt, tag="eb")
            nc.scalar.activation(out=eb, in_=cb_s[s][:, 0:G_TILES], func=Exp)
            eb_s[s] = eb
            tt_s[s] = None

        def stage_a(s):
            at = atp.tile([128, G_TILES, NB_LOC], dt, tag="at")
            bic_b = bic[:, s * G_TILES:(s + 1) * G_TILES].broadcast_to(
                (128, G_TILES, NB_LOC))
            iota_b = bass.AP(
                tensor=iota.tensor, offset=iota.offset,
                ap=[iota.ap[0], [0, G_TILES], iota.ap[1]])
            nc.vector.tensor_tensor(out=at, in0=bic_b, in1=iota_b,
                                    op=is_equal)
            eb_b = eb_s[s].broadcast_to((128, G_TILES, NB_LOC))
            nc.vector.tensor_tensor(out=at, in0=at, in1=eb_b, op=mult)
            at_s[s] = at
            eb_s[s] = None

        def stage_pool(s):
            stg = cb_s[s][0:NB_LOC, G_TILES:G_TILES + 129]
            for t in range(G_TILES):
                nc.tensor.matmul(stg, lhsT=at_s[s][:, t, :],
                                 rhs=vr_s[s][:, t, :],
                                 start=(t == 0),
                                 stop=(t == G_TILES - 1),
                                 skip_group_check=True)
            at_s[s] = None
            vr_s[s] = None

        def stage_copy(s):
            # copy this slab's [NB_LOC,129] stage into its 32-aligned
            # block of the current group tile
            g = stg_grp[(s // 3) % 2]
            p0 = 32 * (s % 3)
            nc.vector.tensor_copy(
                out=g[p0:p0 + NB_LOC, :],
                in_=cb_s[s][0:NB_LOC, G_TILES:G_TILES + 129])
            cb_s[s] = None

        def stage_route(s):
            # after the last slab of group q: route the 3-slab stage
            # block into numg with one matmul
            q = s // 3
            nc.tensor.matmul(numg, lhsT=rmat[:, q, :],
                             rhs=stg_grp[q % 2],
                             start=(q == 0), stop=(q == NQUAD - 1),
                             skip_group_check=True)

        for i in range(NSLAB_ + 2):
            if i < NSLAB_:
                qt_t, kt_t = stage_load(i)
                stage_s(i, qt_t, kt_t)
                stage_tanh(i)
            j = i - 1
            if 0 <= j < NSLAB_:
                stage_beta(j)
                stage_exp(j)
                stage_a(j)
            k = i - 2
            if 0 <= k < NSLAB_:
                stage_pool(k)
                stage_copy(k)
                if k % 3 == 2 or k == NSLAB_ - 1:
                    stage_route(k)

        gc = finp.tile([NB, 1], f32)
        nc.vector.tensor_scalar(out=gc, in0=numg[:, 128:129],
                                scalar1=1e-30, scalar2=None,
                                op0=mybir.AluOpType.max)
        recip = finp.tile([NB, 1], f32)
        nc.vector.reciprocal(out=recip, in_=gc)
        outt = finp.tile([NB, 128], f32)
        nc.vector.tensor_scalar(out=outt, in0=numg[:, 0:128],
                                scalar1=recip, scalar2=None, op0=mult)
        nc.sync.dma_start(out=out_d, in_=outt)

    nc.compile()
    return nc


def _np_dt(dt_name):
    if dt_name == "float32":
        return np.float32
    from concourse import mybir
    return mybir.dt.np(getattr(mybir.dt, dt_name))


def _shard(Q, K, V, bi):
    """Split rows at segment boundaries into N_CORES contiguous shards."""
    N = Q.shape[0]
    seg_starts = np.searchsorted(bi, np.arange(NUM_SEGMENTS + 1)).astype(np.int64)
    split_rows = [0]
    split_segs = [0]
    for m in range(1, N_CORES):
        ideal = (N * m) // N_CORES
        s = int(np.argmin(np.abs(seg_starts - ideal)))
        s = min(max(s, split_segs[-1]), NUM_SEGMENTS)
        split_segs.append(s)
        split_rows.append(int(seg_starts[s]))
    split_segs.append(NUM_SEGMENTS)
    split_rows.append(N)
    return split_rows, split_segs


def kernel(Q, K, V, batch_index, W, U, v):
    from concourse.bass_utils import run_bass_kernel_spmd

    Q = np.asarray(Q, dtype=np.float32)
    K = np.asarray(K, dtype=np.float32)
    V = np.asarray(V, dtype=np.float32)
    W = np.asarray(W, dtype=np.float32)
    U = np.asarray(U, dtype=np.float32)
    v = np.asarray(v, dtype=np.float32)
    bi = np.asarray(batch_index).astype(np.int64)
    N = Q.shape[0]

    ndt = _np_dt(DT_NAME)
    split_rows, split_segs = _shard(Q, K, V, bi)
    max_rm = max(split_rows[m + 1] - split_rows[m] for m in range(N_CORES))
    nslab = max(1, -(-max_rm // SLABW))  # ceil
    R_PAD_ = nslab * SLABW
    T_TILES_ = R_PAD_ // 128

    Wc = W.astype(ndt)
    Uc = U.astype(ndt)
    vvec = v.reshape(128, 1).astype(ndt)

    # --- per-core local-slot bic + routing matrices -------------------
    # Each 1024-row slab of sorted batch_index spans few distinct
    # segments; rows carry a slab-local slot id (0..NB_LOC-1) and a
    # host-built one-hot R maps (slab, slot) -> core-local segment row.
    per_core = []
    nb_loc = 4
    for m in range(N_CORES):
        r0, r1 = split_rows[m], split_rows[m + 1]
        s0 = split_segs[m]
        R_m = r1 - r0
        bl = (bi[r0:r1] - s0).astype(np.int32)
        lo = np.zeros(nslab, np.int32)
        nreal = np.zeros(nslab, np.int32)
        haspad = np.zeros(nslab, np.int32)
        for s in range(nslab):
            a, b = s * SLABW, min((s + 1) * SLABW, R_m)
            if a >= R_m:
                haspad[s] = 1
                continue
            lo[s] = bl[a]
            nreal[s] = bl[b - 1] - bl[a] + 1
            if b < (s + 1) * SLABW:
                haspad[s] = 1
        nb_loc = max(nb_loc, int((nreal + haspad).max()))
        per_core.append((r0, r1, s0, bl, lo, nreal))
    NB_LOC = nb_loc

    iota = np.tile(np.arange(NB_LOC, dtype=np.float32)[None, :],
                   (128, 1)).astype(ndt)

    in_maps = []
    for m in range(N_CORES):
        r0, r1, s0, bl, lo, nreal = per_core[m]
        s1 = split_segs[m + 1]
        R_m = r1 - r0
        nb = s1 - s0
        assert R_m <= R_PAD_, f"core {m}: {R_m} rows > R_PAD {R_PAD_}"
        assert 0 < nb <= NB - 1, f"core {m}: {nb} segments"

        tfull, rem = divmod(R_m, 128)

        qt = np.zeros((128, R_PAD_), dtype=ndt)
        qt[:, :R_m] = Q[r0:r1].T
        kt = np.zeros((128, R_PAD_), dtype=ndt)
        kt[:, :R_m] = K[r0:r1].T

        vr = np.zeros((128, T_TILES_, 129), dtype=ndt)
        vr[:, :, 128] = 1.0
        Vp = V[r0:r1]
        vr[:, :tfull, :128] = Vp[:tfull * 128].reshape(tfull, 128, 128).transpose(1, 0, 2)
        if rem:
            vr[:rem, tfull, :128] = Vp[tfull * 128:]
        vr = vr.reshape(128, T_TILES_ * 129)

        # slab-local slot per row; pad rows take slot nreal[slab]
        slot_pad = np.repeat(nreal, SLABW).astype(np.float32)
        bicv = slot_pad.copy()
        bicv[:R_m] = bl - np.repeat(lo, SLABW)[:R_m]
        bic = bicv.reshape(T_TILES_, 128).T.astype(ndt)

        # routing one-hot: group row 32*(s%3)+j -> lo[s]+j (real) or PAD_SLOT
        import ml_dtypes
        nquad = -(-nslab // 3)
        rmat = np.zeros((128, nquad, NB), dtype=ml_dtypes.bfloat16)
        for s in range(nslab):
            for j in range(NB_LOC):
                g = lo[s] + j if j < nreal[s] else PAD_SLOT
                rmat[32 * (s % 3) + j, s // 3, g] = 1.0
        rmat = rmat.reshape(128, nquad * NB)

        in_maps.append({
            "qt": qt, "kt": kt, "vr": vr, "bic": bic,
            "iota": iota, "rmat": rmat, "w": Wc, "u": Uc, "vv": vvec,
        })

    import os as _os
    key = (DT_NAME, nslab, NB_LOC, _os.environ.get('K_LOADG', '6'),
           _os.environ.get('K_BUFS', '4'))
    if key not in _compiled:
        _compiled[key] = _build_nc(DT_NAME, nslab=nslab, nb_loc=NB_LOC)
    nc = _compiled[key]

    try:
        res = run_bass_kernel_spmd(nc, in_maps, core_ids=list(range(N_CORES)))
    except Exception:
        res = run_bass_kernel_spmd(nc, in_maps, core_ids=list(range(N_CORES)))
    global LAST_RESULT
    LAST_RESULT = res

    out = np.zeros((NUM_SEGMENTS, D), dtype=np.float32)
    for m in range(N_CORES):
        s0, s1 = split_segs[m], split_segs[m + 1]
        out[s0:s1] = res.results[m]["out"][:s1 - s0]
    return out


# revision 24
# speedup vs baseline: 1.0834x; 1.0834x over previous
"""Segment-softmax additive-attention pooling on 8 TRN2 NeuronCores.

Math (per node n with segment b = batch_index[n]):
    beta[n]  = v . tanh(Q@W + K@U)[n]
    alpha[n] = exp(beta[n]) / sum_{m in b} exp(beta[m])
    out[b]   = sum_{n in b} alpha[n] * V[n]

Strategy:
  - batch_index is sorted -> shard rows across 8 cores at segment
    boundaries (no cross-core reduction, <=79 segments per core).
  - Host pre-transposes Q,K to [D, rows], block-transposes V to a
    [128, tiles*129] layout with a column of ones baked in after each
    128-wide V block (gives the softmax denominator for free), and
    reformats batch_index to per-tile f32 columns. Every DMA is then
    fully contiguous per partition and batched to ~2MB transfers.
  - Compute dtype fp16 (same bytes/rate as bf16, 8x the mantissa).
  - On device, per 1024-row slab (software-pipelined across slabs):
      S^T = W^T Q^T + U^T K^T        (PE, 4 matmuls, rhs free=512)
      T^T = tanh(S^T)                (ACT, one op per slab)
      beta = T @ v                   (PE, 8 matmuls of rhs free=1)
      e = exp(beta)                  (ACT)
      A[n,j] = (bi[n]==iota[j]) * e[n]   (DVE, 2 broadcast tensor_tensor)
      NumG[j,:] += A^T @ [V | 1]     (PE, accumulated in one PSUM bank)
  - Final: out[j,:] = NumG[j,0:128] / max(NumG[j,128],1e-30), DMA out.
  - Scatter-add is expressed as matmul with a one-hot-weighted A, so no
    indirect addressing at all; softmax normalization is folded into a
    single division at the end.
"""

import numpy as np

N_CORES = 8
D = 128
NUM_SEGMENTS = 512
SLABW = 1024
NSLAB = 63
R_PAD = NSLAB * SLABW        # 64512 padded rows per core
T_TILES = R_PAD // 128       # 504 tiles of 128 rows
G_TILES = SLABW // 128       # 8 row-tiles per slab
NB = 80                      # local segment slots per core (partition dim)
PAD_SLOT = NB - 1            # local slot that padding rows are routed to

DT_NAME = "float16"          # compute dtype for Q/K/V/W/U/v ("float32"|"bfloat16")

_compiled = {}
LAST_RESULT = None


def _build_nc(dt_name, nslab=NSLAB):
    import concourse.bass as bass
    import concourse.bacc as bacc
    import concourse.tile as tile
    from concourse import mybir

    NSLAB_ = nslab
    R_PAD_ = NSLAB_ * SLABW
    T_TILES_ = R_PAD_ // 128
    # DMA load groups (in slabs): small first groups so compute starts
    # early, small last groups so the compute tail overlaps the final
    # transfers instead of draining serially after the last byte.
    import os as _os
    _lg = int(_os.environ.get("K_LOADG", "6"))
    front = [int(x) for x in _os.environ.get("K_FRONT", "1,1,2").split(",")]
    back = [int(x) for x in _os.environ.get("K_BACK", "2,1,1").split(",")]
    group_sizes = []
    rem = NSLAB_
    for want in front:
        if rem > sum(back):
            g = min(want, rem - sum(back))
            group_sizes.append(g)
            rem -= g
    while rem > sum(back):
        g = min(_lg, rem - sum(back))
        group_sizes.append(g)
        rem -= g
    for want in back:
        if rem > 0:
            g = min(want, rem)
            group_sizes.append(g)
            rem -= g
    assert sum(group_sizes) == NSLAB_, (group_sizes, NSLAB_)
    LOADG_MAX = max(group_sizes)
    group_start = [0]
    for g in group_sizes:
        group_start.append(group_start[-1] + g)

    dt = getattr(mybir.dt, dt_name)
    f32 = mybir.dt.float32
    nc = bacc.Bacc("TRN2", target_bir_lowering=False, debug=False,
                   num_devices=N_CORES)

    qt_d = nc.dram_tensor("qt", [128, R_PAD_], dt, kind="ExternalInput").ap()
    kt_d = nc.dram_tensor("kt", [128, R_PAD_], dt, kind="ExternalInput").ap()
    vr_d = nc.dram_tensor("vr", [128, T_TILES_ * 129], dt, kind="ExternalInput").ap()
    bic_d = nc.dram_tensor("bic", [128, T_TILES_], dt, kind="ExternalInput").ap()
    iota_d = nc.dram_tensor("iota", [128, NB], dt, kind="ExternalInput").ap()
    w_d = nc.dram_tensor("w", [128, 128], dt, kind="ExternalInput").ap()
    u_d = nc.dram_tensor("u", [128, 128], dt, kind="ExternalInput").ap()
    vv_d = nc.dram_tensor("vv", [128, 1], dt, kind="ExternalInput").ap()
    out_d = nc.dram_tensor("out", [NB, 128], f32, kind="ExternalOutput").ap()

    Tanh = mybir.ActivationFunctionType.Tanh
    Exp = mybir.ActivationFunctionType.Exp
    is_equal = mybir.AluOpType.is_equal
    mult = mybir.AluOpType.mult

    _bufs = int(_os.environ.get("K_BUFS", "3"))
    with tile.TileContext(nc) as tc, \
         tc.tile_pool(name="const", bufs=1) as constp, \
         tc.tile_pool(name="qk", bufs=_bufs) as qkp, \
         tc.tile_pool(name="vsl", bufs=_bufs) as vslp, \
         tc.tile_pool(name="tt", bufs=3) as ttp, \
         tc.tile_pool(name="sm", bufs=4) as smp, \
         tc.tile_pool(name="at", bufs=3) as atp, \
         tc.tile_pool(name="fin", bufs=1) as finp, \
         tc.tile_pool(name="ps_s", bufs=2, space="PSUM") as pss, \
         tc.tile_pool(name="ps_b", bufs=2, space="PSUM") as psb, \
         tc.tile_pool(name="ps_o", bufs=1, space="PSUM") as pso:

        wt = constp.tile([128, 128], dt)
        nc.sync.dma_start(out=wt, in_=w_d)
        ut = constp.tile([128, 128], dt)
        nc.sync.dma_start(out=ut, in_=u_d)
        vv = constp.tile([128, 1], dt)
        nc.scalar.dma_start(out=vv, in_=vv_d)
        iota = constp.tile([128, NB], dt)
        nc.scalar.dma_start(out=iota, in_=iota_d)
        bic = constp.tile([128, T_TILES_], dt)
        nc.scalar.dma_start(out=bic, in_=bic_d)

        numg = pso.tile([NB, 129], f32)

        # pipeline state per slab
        vr_s = [None] * NSLAB_
        st_s = [None] * NSLAB_
        tt_s = [None] * NSLAB_
        bp_s = [None] * NSLAB_
        eb_s = [None] * NSLAB_
        at_s = [None] * NSLAB_

        qt_g = [None]
        kt_g = [None]
        vr_g = [None]
        g_base = [0]

        def stage_load_group(g):
            s0 = group_start[g]
            ns = group_sizes[g]
            w0 = s0 * SLABW
            w1 = w0 + ns * SLABW
            qt_t = qkp.tile([128, LOADG_MAX * SLABW], dt, tag="qt")
            nc.sync.dma_start(out=qt_t[:, :w1 - w0], in_=qt_d[:, w0:w1])
            kt_t = qkp.tile([128, LOADG_MAX * SLABW], dt, tag="kt")
            nc.sync.dma_start(out=kt_t[:, :w1 - w0], in_=kt_d[:, w0:w1])
            t0 = s0 * G_TILES
            t1 = t0 + ns * G_TILES
            vr_t = vslp.tile([128, LOADG_MAX * G_TILES, 129], dt, tag="vr")
            nc.sync.dma_start(
                out=vr_t[:, :t1 - t0, :],
                in_=vr_d[:, t0 * 129:t1 * 129].rearrange(
                    "p (t d) -> p t d", d=129))
            qt_g[0], kt_g[0], vr_g[0] = qt_t, kt_t, vr_t
            g_base[0] = s0

        next_group = [0]

        def stage_load(s):
            if next_group[0] < len(group_start) - 1 and s == group_start[next_group[0]]:
                stage_load_group(next_group[0])
                next_group[0] += 1
            o = (s - g_base[0]) * SLABW
            vr_s[s] = vr_g[0][:, (s - g_base[0]) * G_TILES:
                              (s - g_base[0] + 1) * G_TILES, :]
            return (qt_g[0][:, o:o + SLABW], kt_g[0][:, o:o + SLABW])

        def stage_s(s, qt_t, kt_t):
            st = pss.tile([128, SLABW], f32, tag="st")
            for h in range(SLABW // 512):
                sl = slice(h * 512, (h + 1) * 512)
                nc.tensor.matmul(st[:, sl], lhsT=wt, rhs=qt_t[:, sl],
                                 start=True, stop=False)
                nc.tensor.matmul(st[:, sl], lhsT=ut, rhs=kt_t[:, sl],
                                 start=False, stop=True)
            st_s[s] = st

        def stage_tanh(s):
            tt = ttp.tile([128, SLABW], dt, tag="tt")
            nc.scalar.activation(out=tt, in_=st_s[s], func=Tanh)
            tt_s[s] = tt
            st_s[s] = None

        def stage_beta(s):
            bp = psb.tile([128, G_TILES], f32, tag="bp")
            tt = tt_s[s]
            for t in range(G_TILES):
                nc.tensor.matmul(bp[:, t:t + 1],
                                 lhsT=tt[:, t * 128:(t + 1) * 128],
                                 rhs=vv, start=True, stop=True)
            bp_s[s] = bp

        def stage_exp(s):
            eb = smp.tile([128, G_TILES], dt, tag="eb")
            nc.scalar.activation(out=eb, in_=bp_s[s], func=Exp)
            eb_s[s] = eb
            bp_s[s] = None
            tt_s[s] = None

        def stage_a(s):
            at = atp.tile([128, G_TILES, NB], dt, tag="at")
            bic_b = bic[:, s * G_TILES:(s + 1) * G_TILES].broadcast_to(
                (128, G_TILES, NB))
            iota_b = bass.AP(
                tensor=iota.tensor, offset=iota.offset,
                ap=[iota.ap[0], [0, G_TILES], iota.ap[1]])
            nc.vector.tensor_tensor(out=at, in0=bic_b, in1=iota_b,
                                    op=is_equal)
            eb_b = eb_s[s].broadcast_to((128, G_TILES, NB))
            nc.vector.tensor_tensor(out=at, in0=at, in1=eb_b, op=mult)
            at_s[s] = at
            eb_s[s] = None

        def stage_pool(s):
            for t in range(G_TILES):
                g = s * G_TILES + t
                nc.tensor.matmul(numg, lhsT=at_s[s][:, t, :],
                                 rhs=vr_s[s][:, t, :],
                                 start=(g == 0),
                                 stop=(g == NSLAB_ * G_TILES - 1),
                                 skip_group_check=True)
            at_s[s] = None
            vr_s[s] = None

        for i in range(NSLAB_ + 2):
            if i < NSLAB_:
                qt_t, kt_t = stage_load(i)
                stage_s(i, qt_t, kt_t)
                stage_tanh(i)
            j = i - 1
            if 0 <= j < NSLAB_:
                stage_beta(j)
                stage_exp(j)
                stage_a(j)
            k = i - 2
            if 0 <= k < NSLAB_:
                stage_pool(k)

        gc = finp.tile([NB, 1], f32)
        nc.vector.tensor_scalar(out=gc, in0=numg[:, 128:129],
                                scalar1=1e-30, scalar2=None,
                                op0=mybir.AluOpType.max)
        recip = finp.tile([NB, 1], f32)
        nc.vector.reciprocal(out=recip, in_=gc)
        outt = finp.tile([NB, 128], f32)
        nc.vector.tensor_scalar(out=outt, in0=numg[:, 0:128],
                                scalar1=recip, scalar2=None, op0=mult)
        nc.sync.dma_start(out=out_d, in_=outt)

    nc.compile()
    return nc


def _np_dt(dt_name):
    if dt_name == "float32":
        return np.float32
    from concourse import mybir
    return mybir.dt.np(getattr(mybir.dt, dt_name))


def _shard(Q, K, V, bi):
    """Split rows at segment boundaries into N_CORES contiguous shards."""
    N = Q.shape[0]
    seg_starts = np.searchsorted(bi, np.arange(NUM_SEGMENTS + 1)).astype(np.int64)
    split_rows = [0]
    split_segs = [0]
    for m in range(1, N_CORES):
        ideal = (N * m) // N_CORES
        s = int(np.argmin(np.abs(seg_starts - ideal)))
        s = min(max(s, split_segs[-1]), NUM_SEGMENTS)
        split_segs.append(s)
        split_rows.append(int(seg_starts[s]))
    split_segs.append(NUM_SEGMENTS)
    split_rows.append(N)
    return split_rows, split_segs


def kernel(Q, K, V, batch_index, W, U, v):
    from concourse.bass_utils import run_bass_kernel_spmd

    Q = np.asarray(Q, dtype=np.float32)
    K = np.asarray(K, dtype=np.float32)
    V = np.asarray(V, dtype=np.float32)
    W = np.asarray(W, dtype=np.float32)
    U = np.asarray(U, dtype=np.float32)
    v = np.asarray(v, dtype=np.float32)
    bi = np.asarray(batch_index).astype(np.int64)
    N = Q.shape[0]

    ndt = _np_dt(DT_NAME)
    split_rows, split_segs = _shard(Q, K, V, bi)
    max_rm = max(split_rows[m + 1] - split_rows[m] for m in range(N_CORES))
    nslab = max(1, -(-max_rm // SLABW))  # ceil
    R_PAD_ = nslab * SLABW
    T_TILES_ = R_PAD_ // 128

    iota = np.tile(np.arange(NB, dtype=np.float32)[None, :], (128, 1)).astype(ndt)
    Wc = W.astype(ndt)
    Uc = U.astype(ndt)
    vvec = v.reshape(128, 1).astype(ndt)

    in_maps = []
    for m in range(N_CORES):
        r0, r1 = split_rows[m], split_rows[m + 1]
        s0, s1 = split_segs[m], split_segs[m + 1]
        R_m = r1 - r0
        nb = s1 - s0
        assert R_m <= R_PAD_, f"core {m}: {R_m} rows > R_PAD {R_PAD_}"
        assert 0 < nb <= NB - 1, f"core {m}: {nb} segments"

        tfull, rem = divmod(R_m, 128)

        qt = np.zeros((128, R_PAD_), dtype=ndt)
        qt[:, :R_m] = Q[r0:r1].T
        kt = np.zeros((128, R_PAD_), dtype=ndt)
        kt[:, :R_m] = K[r0:r1].T

        vr = np.zeros((128, T_TILES_, 129), dtype=ndt)
        vr[:, :, 128] = 1.0
        Vp = V[r0:r1]
        vr[:, :tfull, :128] = Vp[:tfull * 128].reshape(tfull, 128, 128).transpose(1, 0, 2)
        if rem:
            vr[:rem, tfull, :128] = Vp[tfull * 128:]
        vr = vr.reshape(128, T_TILES_ * 129)

        bic = np.full((128, T_TILES_), float(PAD_SLOT), dtype=np.float32)
        bl = (bi[r0:r1] - s0).astype(np.float32)
        bic[:, :tfull] = bl[:tfull * 128].reshape(tfull, 128).T
        if rem:
            bic[:rem, tfull] = bl[tfull * 128:]
        bic = bic.astype(ndt)

        in_maps.append({
            "qt": qt, "kt": kt, "vr": vr, "bic": bic,
            "iota": iota, "w": Wc, "u": Uc, "vv": vvec,
        })

    import os as _os
    key = (DT_NAME, nslab, _os.environ.get('K_LOADG', '6'),
           _os.environ.get('K_BUFS', '3'),
           _os.environ.get('K_FRONT', '1,1,2'),
           _os.environ.get('K_BACK', '2,1,1'))
    if key not in _compiled:
        _compiled[key] = _build_nc(DT_NAME, nslab=nslab)
    nc = _compiled[key]

    try:
        res = run_bass_kernel_spmd(nc, in_maps, core_ids=list(range(N_CORES)))
    except Exception:
        res = run_bass_kernel_spmd(nc, in_maps, core_ids=list(range(N_CORES)))
    global LAST_RESULT
    LAST_RESULT = res

    out = np.zeros((NUM_SEGMENTS, D), dtype=np.float32)
    for m in range(N_CORES):
        s0, s1 = split_segs[m], split_segs[m + 1]
        out[s0:s1] = res.results[m]["out"][:s1 - s0]
    return out


# revision 27
# speedup vs baseline: 1.0986x; 1.0140x over previous
"""Segment-softmax additive-attention pooling on 8 TRN2 NeuronCores.

Math (per node n with segment b = batch_index[n]):
    beta[n]  = v . tanh(Q@W + K@U)[n]
    alpha[n] = exp(beta[n]) / sum_{m in b} exp(beta[m])
    out[b]   = sum_{n in b} alpha[n] * V[n]

Strategy:
  - batch_index is sorted -> shard rows across 8 cores at segment
    boundaries (no cross-core reduction, <=79 segments per core).
  - Host pre-transposes Q,K to [D, rows], block-transposes V to a
    [128, tiles*129] layout with a column of ones baked in after each
    128-wide V block (gives the softmax denominator for free), and
    reformats batch_index to per-tile f32 columns. Every DMA is then
    fully contiguous per partition and batched to ~2MB transfers.
  - Compute dtype fp16 (same bytes/rate as bf16, 8x the mantissa).
  - On device, per 1024-row slab (software-pipelined across slabs):
      S^T = W^T Q^T + U^T K^T        (PE, 4 matmuls, rhs free=512)
      T^T = tanh(S^T)                (ACT, one op per slab)
      beta = T @ v                   (PE, 8 matmuls of rhs free=1)
      e = exp(beta)                  (ACT)
      A[n,j] = (bi[n]==iota[j]) * e[n]   (DVE, 2 broadcast tensor_tensor)
      NumG[j,:] += A^T @ [V | 1]     (PE, accumulated in one PSUM bank)
  - Final: out[j,:] = NumG[j,0:128] / max(NumG[j,128],1e-30), DMA out.
  - Scatter-add is expressed as matmul with a one-hot-weighted A, so no
    indirect addressing at all; softmax normalization is folded into a
    single division at the end.
"""

import numpy as np

N_CORES = 8
D = 128
NUM_SEGMENTS = 512
SLABW = 1024
NSLAB = 63
R_PAD = NSLAB * SLABW        # 64512 padded rows per core
T_TILES = R_PAD // 128       # 504 tiles of 128 rows
G_TILES = SLABW // 128       # 8 row-tiles per slab
NB = 80                      # local segment slots per core (partition dim)
PAD_SLOT = NB - 1            # local slot that padding rows are routed to

DT_NAME = "float16"          # compute dtype for Q/K/V/W/U/v ("float32"|"bfloat16")

_compiled = {}
LAST_RESULT = None


def _build_nc(dt_name, nslab=NSLAB):
    import concourse.bass as bass
    import concourse.bacc as bacc
    import concourse.tile as tile
    from concourse import mybir

    NSLAB_ = nslab
    R_PAD_ = NSLAB_ * SLABW
    T_TILES_ = R_PAD_ // 128
    # DMA load groups (in slabs): small first groups so compute starts
    # early, small last groups so the compute tail overlaps the final
    # transfers instead of draining serially after the last byte.
    import os as _os
    _lg = int(_os.environ.get("K_LOADG", "8"))
    front = [int(x) for x in _os.environ.get("K_FRONT", "1,2,4").split(",")]
    back = [int(x) for x in _os.environ.get("K_BACK", "2,1").split(",")]
    group_sizes = []
    rem = NSLAB_
    for want in front:
        if rem > sum(back):
            g = min(want, rem - sum(back))
            group_sizes.append(g)
            rem -= g
    while rem > sum(back):
        g = min(_lg, rem - sum(back))
        group_sizes.append(g)
        rem -= g
    for want in back:
        if rem > 0:
            g = min(want, rem)
            group_sizes.append(g)
            rem -= g
    assert sum(group_sizes) == NSLAB_, (group_sizes, NSLAB_)
    LOADG_MAX = max(group_sizes)
    group_start = [0]
    for g in group_sizes:
        group_start.append(group_start[-1] + g)

    dt = getattr(mybir.dt, dt_name)
    f32 = mybir.dt.float32
    nc = bacc.Bacc("TRN2", target_bir_lowering=False, debug=False,
                   num_devices=N_CORES)

    qt_d = nc.dram_tensor("qt", [128, R_PAD_], dt, kind="ExternalInput").ap()
    kt_d = nc.dram_tensor("kt", [128, R_PAD_], dt, kind="ExternalInput").ap()
    vr_d = nc.dram_tensor("vr", [128, T_TILES_ * 129], dt, kind="ExternalInput").ap()
    bic_d = nc.dram_tensor("bic", [128, T_TILES_], dt, kind="ExternalInput").ap()
    iota_d = nc.dram_tensor("iota", [128, NB], dt, kind="ExternalInput").ap()
    w_d = nc.dram_tensor("w", [128, 128], dt, kind="ExternalInput").ap()
    u_d = nc.dram_tensor("u", [128, 128], dt, kind="ExternalInput").ap()
    vv_d = nc.dram_tensor("vv", [128, 1], dt, kind="ExternalInput").ap()
    out_d = nc.dram_tensor("out", [NB, 128], f32, kind="ExternalOutput").ap()

    Tanh = mybir.ActivationFunctionType.Tanh
    Exp = mybir.ActivationFunctionType.Exp
    is_equal = mybir.AluOpType.is_equal
    mult = mybir.AluOpType.mult

    _bufs = int(_os.environ.get("K_BUFS", "3"))
    with tile.TileContext(nc) as tc, \
         tc.tile_pool(name="const", bufs=1) as constp, \
         tc.tile_pool(name="qk", bufs=_bufs) as qkp, \
         tc.tile_pool(name="vsl", bufs=_bufs) as vslp, \
         tc.tile_pool(name="tt", bufs=3) as ttp, \
         tc.tile_pool(name="sm", bufs=4) as smp, \
         tc.tile_pool(name="at", bufs=3) as atp, \
         tc.tile_pool(name="fin", bufs=1) as finp, \
         tc.tile_pool(name="ps_s", bufs=2, space="PSUM") as pss, \
         tc.tile_pool(name="ps_b", bufs=2, space="PSUM") as psb, \
         tc.tile_pool(name="ps_o", bufs=1, space="PSUM") as pso:

        wt = constp.tile([128, 128], dt)
        nc.sync.dma_start(out=wt, in_=w_d)
        ut = constp.tile([128, 128], dt)
        nc.sync.dma_start(out=ut, in_=u_d)
        vv = constp.tile([128, 1], dt)
        nc.scalar.dma_start(out=vv, in_=vv_d)
        iota = constp.tile([128, NB], dt)
        nc.scalar.dma_start(out=iota, in_=iota_d)
        bic = constp.tile([128, T_TILES_], dt)
        nc.scalar.dma_start(out=bic, in_=bic_d)

        numg = pso.tile([NB, 129], f32)

        # pipeline state per slab
        vr_s = [None] * NSLAB_
        st_s = [None] * NSLAB_
        tt_s = [None] * NSLAB_
        bp_s = [None] * NSLAB_
        eb_s = [None] * NSLAB_
        at_s = [None] * NSLAB_

        qt_g = [None]
        kt_g = [None]
        vr_g = [None]
        g_base = [0]

        def stage_load_group(g):
            s0 = group_start[g]
            ns = group_sizes[g]
            w0 = s0 * SLABW
            w1 = w0 + ns * SLABW
            qt_t = qkp.tile([128, LOADG_MAX * SLABW], dt, tag="qt")
            nc.sync.dma_start(out=qt_t[:, :w1 - w0], in_=qt_d[:, w0:w1])
            kt_t = qkp.tile([128, LOADG_MAX * SLABW], dt, tag="kt")
            nc.sync.dma_start(out=kt_t[:, :w1 - w0], in_=kt_d[:, w0:w1])
            t0 = s0 * G_TILES
            t1 = t0 + ns * G_TILES
            vr_t = vslp.tile([128, LOADG_MAX * G_TILES, 129], dt, tag="vr")
            nc.sync.dma_start(
                out=vr_t[:, :t1 - t0, :],
                in_=vr_d[:, t0 * 129:t1 * 129].rearrange(
                    "p (t d) -> p t d", d=129))
            qt_g[0], kt_g[0], vr_g[0] = qt_t, kt_t, vr_t
            g_base[0] = s0

        next_group = [0]

        def stage_load(s):
            if next_group[0] < len(group_start) - 1 and s == group_start[next_group[0]]:
                stage_load_group(next_group[0])
                next_group[0] += 1
            o = (s - g_base[0]) * SLABW
            vr_s[s] = vr_g[0][:, (s - g_base[0]) * G_TILES:
                              (s - g_base[0] + 1) * G_TILES, :]
            return (qt_g[0][:, o:o + SLABW], kt_g[0][:, o:o + SLABW])

        def stage_s(s, qt_t, kt_t):
            st = pss.tile([128, SLABW], f32, tag="st")
            for h in range(SLABW // 512):
                sl = slice(h * 512, (h + 1) * 512)
                nc.tensor.matmul(st[:, sl], lhsT=wt, rhs=qt_t[:, sl],
                                 start=True, stop=False)
                nc.tensor.matmul(st[:, sl], lhsT=ut, rhs=kt_t[:, sl],
                                 start=False, stop=True)
            st_s[s] = st

        def stage_tanh(s):
            tt = ttp.tile([128, SLABW], dt, tag="tt")
            nc.scalar.activation(out=tt, in_=st_s[s], func=Tanh)
            tt_s[s] = tt
            st_s[s] = None

        def stage_beta(s):
            bp = psb.tile([128, G_TILES], f32, tag="bp")
            tt = tt_s[s]
            for t in range(G_TILES):
                nc.tensor.matmul(bp[:, t:t + 1],
                                 lhsT=tt[:, t * 128:(t + 1) * 128],
                                 rhs=vv, start=True, stop=True)
            bp_s[s] = bp

        def stage_exp(s):
            eb = smp.tile([128, G_TILES], dt, tag="eb")
            nc.scalar.activation(out=eb, in_=bp_s[s], func=Exp)
            eb_s[s] = eb
            bp_s[s] = None
            tt_s[s] = None

        def stage_a(s):
            at = atp.tile([128, G_TILES, NB], dt, tag="at")
            bic_b = bic[:, s * G_TILES:(s + 1) * G_TILES].broadcast_to(
                (128, G_TILES, NB))
            iota_b = bass.AP(
                tensor=iota.tensor, offset=iota.offset,
                ap=[iota.ap[0], [0, G_TILES], iota.ap[1]])
            nc.vector.tensor_tensor(out=at, in0=bic_b, in1=iota_b,
                                    op=is_equal)
            eb_b = eb_s[s].broadcast_to((128, G_TILES, NB))
            nc.vector.tensor_tensor(out=at, in0=at, in1=eb_b, op=mult)
            at_s[s] = at
            eb_s[s] = None

        def stage_pool(s):
            for t in range(G_TILES):
                g = s * G_TILES + t
                nc.tensor.matmul(numg, lhsT=at_s[s][:, t, :],
                                 rhs=vr_s[s][:, t, :],
                                 start=(g == 0),
                                 stop=(g == NSLAB_ * G_TILES - 1),
                                 skip_group_check=True)
            at_s[s] = None
            vr_s[s] = None

        for i in range(NSLAB_ + 2):
            if i < NSLAB_:
                qt_t, kt_t = stage_load(i)
                stage_s(i, qt_t, kt_t)
                stage_tanh(i)
            j = i - 1
            if 0 <= j < NSLAB_:
                stage_beta(j)
                stage_exp(j)
                stage_a(j)
            k = i - 2
            if 0 <= k < NSLAB_:
                stage_pool(k)

        gc = finp.tile([NB, 1], f32)
        nc.vector.tensor_scalar(out=gc, in0=numg[:, 128:129],
                                scalar1=1e-30, scalar2=None,
                                op0=mybir.AluOpType.max)
        recip = finp.tile([NB, 1], f32)
        nc.vector.reciprocal(out=recip, in_=gc)
        outt = finp.tile([NB, 128], f32)
        nc.vector.tensor_scalar(out=outt, in0=numg[:, 0:128],
                                scalar1=recip, scalar2=None, op0=mult)
        nc.sync.dma_start(out=out_d, in_=outt)

    nc.compile()
    return nc


def _np_dt(dt_name):
    if dt_name == "float32":
        return np.float32
    from concourse import mybir
    return mybir.dt.np(getattr(mybir.dt, dt_name))


def _shard(Q, K, V, bi):
    """Split rows at segment boundaries into N_CORES contiguous shards,
    minimizing the max shard size (which sets the padded slab count)."""
    N = Q.shape[0]
    seg_starts = np.searchsorted(bi, np.arange(NUM_SEGMENTS + 1)).astype(np.int64)

    def greedy(target):
        # ordered partition with per-shard capacity `target`
        segs = [0]
        while segs[-1] < NUM_SEGMENTS and len(segs) <= N_CORES:
            limit = seg_starts[segs[-1]] + target
            s = int(np.searchsorted(seg_starts, limit, side="right")) - 1
            if s <= segs[-1]:
                return None  # a single segment exceeds target
            segs.append(min(s, NUM_SEGMENTS))
        if segs[-1] < NUM_SEGMENTS:
            return None      # needs more than N_CORES shards
        return segs

    # binary search the smallest feasible max-shard-size
    lo_t, hi_t = N // N_CORES, N
    while lo_t < hi_t:
        mid = (lo_t + hi_t) // 2
        if greedy(mid) is not None:
            hi_t = mid
        else:
            lo_t = mid + 1
    segs = greedy(lo_t)
    # pad out to exactly N_CORES shards (steal one trailing segment each)
    while len(segs) < N_CORES + 1:
        segs.insert(-1, segs[-2])
    for m in range(N_CORES, 0, -1):
        if segs[m] <= segs[m - 1]:
            segs[m - 1] = segs[m] - 1
    split_segs = segs
    split_rows = [int(seg_starts[s]) for s in split_segs]
    return split_rows, split_segs


def kernel(Q, K, V, batch_index, W, U, v):
    from concourse.bass_utils import run_bass_kernel_spmd

    Q = np.asarray(Q, dtype=np.float32)
    K = np.asarray(K, dtype=np.float32)
    V = np.asarray(V, dtype=np.float32)
    W = np.asarray(W, dtype=np.float32)
    U = np.asarray(U, dtype=np.float32)
    v = np.asarray(v, dtype=np.float32)
    bi = np.asarray(batch_index).astype(np.int64)
    N = Q.shape[0]

    ndt = _np_dt(DT_NAME)
    split_rows, split_segs = _shard(Q, K, V, bi)
    max_rm = max(split_rows[m + 1] - split_rows[m] for m in range(N_CORES))
    nslab = max(1, -(-max_rm // SLABW))  # ceil
    R_PAD_ = nslab * SLABW
    T_TILES_ = R_PAD_ // 128

    iota = np.tile(np.arange(NB, dtype=np.float32)[None, :], (128, 1)).astype(ndt)
    Wc = W.astype(ndt)
    Uc = U.astype(ndt)
    vvec = v.reshape(128, 1).astype(ndt)

    in_maps = []
    for m in range(N_CORES):
        r0, r1 = split_rows[m], split_rows[m + 1]
        s0, s1 = split_segs[m], split_segs[m + 1]
        R_m = r1 - r0
        nb = s1 - s0
        assert R_m <= R_PAD_, f"core {m}: {R_m} rows > R_PAD {R_PAD_}"
        assert 0 < nb <= NB - 1, f"core {m}: {nb} segments"

        tfull, rem = divmod(R_m, 128)

        qt = np.zeros((128, R_PAD_), dtype=ndt)
        qt[:, :R_m] = Q[r0:r1].T
        kt = np.zeros((128, R_PAD_), dtype=ndt)
        kt[:, :R_m] = K[r0:r1].T

        vr = np.zeros((128, T_TILES_, 129), dtype=ndt)
        vr[:, :, 128] = 1.0
        Vp = V[r0:r1]
        vr[:, :tfull, :128] = Vp[:tfull * 128].reshape(tfull, 128, 128).transpose(1, 0, 2)
        if rem:
            vr[:rem, tfull, :128] = Vp[tfull * 128:]
        vr = vr.reshape(128, T_TILES_ * 129)

        bic = np.full((128, T_TILES_), float(PAD_SLOT), dtype=np.float32)
        bl = (bi[r0:r1] - s0).astype(np.float32)
        bic[:, :tfull] = bl[:tfull * 128].reshape(tfull, 128).T
        if rem:
            bic[:rem, tfull] = bl[tfull * 128:]
        bic = bic.astype(ndt)

        in_maps.append({
            "qt": qt, "kt": kt, "vr": vr, "bic": bic,
            "iota": iota, "w": Wc, "u": Uc, "vv": vvec,
        })

    import os as _os
    key = (DT_NAME, nslab, _os.environ.get('K_LOADG', '8'),
           _os.environ.get('K_BUFS', '3'),
           _os.environ.get('K_FRONT', '1,2,4'),
           _os.environ.get('K_BACK', '2,1'))
    if key not in _compiled:
        _compiled[key] = _build_nc(DT_NAME, nslab=nslab)
    nc = _compiled[key]

    try:
        res = run_bass_kernel_spmd(nc, in_maps, core_ids=list(range(N_CORES)))
    except Exception:
        res = run_bass_kernel_spmd(nc, in_maps, core_ids=list(range(N_CORES)))
    global LAST_RESULT
    LAST_RESULT = res

    out = np.zeros((NUM_SEGMENTS, D), dtype=np.float32)
    for m in range(N_CORES):
        s0, s1 = split_segs[m], split_segs[m + 1]
        out[s0:s1] = res.results[m]["out"][:s1 - s0]
    return out
